# revision 1
# baseline (speedup 1.0000x reference)
"""Single-head attention (B=4, N=2048, D=1024) on 8 Trainium2 NeuronCores.

Sharding: core c handles batch c//2 and KEY half c%2, with the duplicated
Q projection eliminated by a pairwise AllGather.  Each core receives only
its key-half of x (xTk, 2MB) plus the three weights; it computes K/V
projections and Q for its OWN 1024 queries (= its key half), then
AllGathers the pair's qT shards through DRAM bounce buffers while the
remaining K/V projection work hides the collective's ~27us fixed latency.
Scores/AV then run over its 1024 keys x all 2048 queries (global order),
producing the partial (unnormalized) attention output and partial softmax
denominator.  The host combines halves: out = (oA + oB) / (dA + dB).

Precision: projections/AV in bf16 (fp32 PSUM accumulation).  The scores
contraction is split: e-blocks 0-3 bf16, e-blocks 4-7 fp8e4 DoubleRow
(two contraction rows per PE cell, 2x bf16 throughput measured).  This
half-fp8 split measures rel err ~1.25e-2 against the f32 reference (full
fp8 was 1.89e-2 — too close to the 2e-2 gate).  exp in fp32 on the
scalar engine; unnormalized softmax (no max subtraction) is safe since
|scores/sqrt(D)| is ~N(0, 0.33^2).  Partial outputs are stored bf16.
"""

from contextlib import ExitStack

import ml_dtypes
import numpy as np

import concourse.bass as bass
import concourse.mybir as mybir
import concourse.tile as tile
from concourse.bass_utils import run_bass_kernel_spmd

B, N, D = 4, 2048, 1024
NCORES = 8
P = 128
NQ = N            # total queries per batch (gathered)
NKH = N // 2      # keys (and local queries) per core
DC = D // P       # 8 contraction chunks
EC = D // P       # 8 embed blocks
JB = NKH // P     # 8 key blocks
F = 512           # matmul moving free dim (one PSUM bank of fp32)
SCALE = 1.0 / np.sqrt(D)
N_WARM_PRE = 5    # dummy matmuls bridging trigger latency until chunk 0
                  # lands; the chunk-gated loop itself (cold-rate work >
                  # arrival pace) then sustains the HAM warm-up window
NBF = 2           # scores e-blocks 0..NBF-1 in bf16; the rest fp8 DoubleRow

BF = mybir.dt.bfloat16
F8 = mybir.dt.float8e4
F32 = mybir.dt.float32

REPLICA_GROUPS = [[0, 1], [2, 3], [4, 5], [6, 7]]

QBF_B = NBF * NKH * 2                 # bytes of the bf16 qT/kT half: 8192
QRANK_B = QBF_B + (EC - NBF) * NKH    # bytes per rank shard: 12288


def _attention_kernel(ctx, tc, out, xTk, wqT, wkT, wvT):
    nc = tc.nc

    consts = ctx.enter_context(tc.tile_pool(name="consts", bufs=1))
    psmain = ctx.enter_context(tc.tile_pool(name="psmain", bufs=2, space="PSUM"))
    psav = ctx.enter_context(tc.tile_pool(name="psav", bufs=6, space="PSUM"))
    outp_big = ctx.enter_context(tc.tile_pool(name="outp_big", bufs=1))
    outp_sm = ctx.enter_context(tc.tile_pool(name="outp_sm", bufs=3))
    small = ctx.enter_context(tc.tile_pool(name="small", bufs=2))
    dram = ctx.enter_context(tc.tile_pool(name="dram", bufs=1, space="DRAM"))

    # Resident SBUF tensors.  qT/kT/qTloc are byte-granular tiles holding
    # a bf16 half (e-blocks 0-3) and an fp8 half (e-blocks 4-7) exposed
    # through bitcast views, so every gather hop is ONE DMA.  qT is
    # rank-major: rank r's shard is a contiguous per-partition byte range.
    xTk_sb = consts.tile([P, DC, NKH], BF, tag="xTk")    # [p, d-chunk, key]
    wkv_sb = consts.tile([P, 2 * DC * D], BF, tag="wkv")
    wk_sb = wkv_sb.rearrange("p (two c e) -> p two c e", two=2, c=DC)[:, 0]
    wv_sb = wkv_sb.rearrange("p (two c e) -> p two c e", two=2, c=DC)[:, 1]
    wq_sb = consts.tile([P, DC, D], BF, tag="wq")
    qT_sb = consts.tile([P, 2, QRANK_B], F8, tag="qT")
    qTloc_sb = consts.tile([P, QRANK_B], F8, tag="qTloc")
    kT_sb = consts.tile([P, QRANK_B], F8, tag="kT")
    v_sb = consts.tile([P, JB, D], BF, tag="v")          # [p, key-block, e]
    pT_sb = consts.tile([P, JB, NQ], BF, tag="pT")       # [p, key-block, query]
    ones_sb = consts.tile([P, 1], BF, tag="ones")

    def _views(t):  # byte range -> (bf16 [P,NBF,NKH], fp8 [P,EC-NBF,NKH])
        bf = t[:, 0:QBF_B].bitcast(BF).rearrange("p (e j) -> p e j", e=NBF)
        f8 = t[:, QBF_B:QRANK_B].rearrange("p (e j) -> p e j", e=EC - NBF)
        return bf, f8

    qTloc_bf, qTloc_f8 = _views(qTloc_sb)
    kT_bf, kT_f8 = _views(kT_sb)
    qT_rk = [_views(qT_sb[:, r, :]) for r in range(2)]

    # DRAM bounce buffers for the pairwise qT AllGather (mixed payload:
    # 1.5MB out, 3MB back).
    cc_in = dram.tile([P, QRANK_B], F8, name="cc_in")
    cc_out = dram.tile([2, P, QRANK_B], F8, name="cc_out")

    nc.vector.memset(ones_sb, 1.0)

    xTr = xTk.rearrange("(c p) j -> p c j", p=P)
    wqr = wqT.rearrange("(c p) e -> p c e", p=P)
    wkr = wkT.rearrange("(c p) e -> p c e", p=P)
    wvr = wvT.rearrange("(c p) e -> p c e", p=P)

    # Input DMAs.  The per-core HBM read port (~358 GB/s) is the early
    # bottleneck: 8MB of input takes ~22us to land.  Phase 1a needs
    # wk + xTk (4MB) chunk-by-chunk ASAP, then wq chunks feed phase
    # 2a-local; wv (needed last, ~60us) follows as one large DMA.
    # Per-queue FIFO on the HWDGE queues preserves this priority.
    xk_dmas = []
    wk_dmas = []
    for c in range(DC):
        xk_dmas.append(nc.sync.dma_start(out=xTk_sb[:, c, :], in_=xTr[:, c, :]))
        wk_dmas.append(nc.sync.dma_start(out=wk_sb[:, c, :], in_=wkr[:, c, :]))
    wq_dmas = []
    for c in range(DC):
        wq_dmas.append(nc.sync.dma_start(out=wq_sb[:, c, :], in_=wqr[:, c, :]))
    wv_dma = nc.sync.dma_start(out=wv_sb[:, :, :], in_=wvr)
    in_dmas = xk_dmas + wk_dmas + wq_dmas + [wv_dma]

    def sp_observe(inst, why):
        n = nc.sync.nop(hint="observe")
        tile.add_dep_helper(n.ins, inst.ins, reason=why)

    # One PSUM tile for dummy warm-up and touch matmuls.  It comes from the
    # psav pool, which no DVE copy reads until phase 2c — so every write to
    # it is PE-local and touch matmuls carry exactly one (DMA) wait.
    warm_src = small.tile([P, 640], BF, tag="warm")
    nc.vector.memset(warm_src, 0.0)
    warm_ps = psav.tile([P, F], F32, tag="po")

    def dummy():
        nc.tensor.matmul(
            warm_ps, lhsT=warm_src[:, 0:P], rhs=warm_src[:, P : P + F],
            start=True, stop=True,
        )

    def touch(t):
        # Trivial matmul whose only purpose is to make the PE observe t's
        # producer (single sync wait), so later real matmuls need none.
        nc.tensor.matmul(
            warm_ps[0:1, 0:1], lhsT=t[:, 0:1], rhs=t[:, 0:1], start=True, stop=True
        )

    # Solid warm-up block: HAM un-throttles only after a ~3.4us window of
    # SUSTAINED PE activity; scattered chunk-gated matmuls never produce one.
    for _ in range(N_WARM_PRE):
        dummy()

    def kT_out(e, jt):
        if e < NBF:
            return kT_bf[:, e, jt * F : (jt + 1) * F]
        return kT_f8[:, e - NBF, jt * F : (jt + 1) * F]

    # Phase 1a e=0: kT[0, j] — chunk-major, gated on each (xk, wk) chunk
    # pair as it lands (~1.4us apart at HBM rate), with touch matmuls
    # carrying the DMA waits and interleaved dummies keeping the PE duty
    # high so HAM stays warm.
    ps0 = psmain.tile([P, F], F32, tag="ps")
    ps1 = psmain.tile([P, F], F32, tag="ps")
    for c in range(DC):
        touch(xTk_sb[:, c, :])
        touch(wk_sb[:, c, :])
        nc.tensor.matmul(
            ps0, lhsT=wk_sb[:, c, 0:P], rhs=xTk_sb[:, c, 0:F],
            start=(c == 0), stop=(c == DC - 1),
        )
        nc.tensor.matmul(
            ps1, lhsT=wk_sb[:, c, 0:P], rhs=xTk_sb[:, c, F : 2 * F],
            start=(c == 0), stop=(c == DC - 1),
        )
        dummy()
        dummy()
    nc.vector.tensor_copy(out=kT_out(0, 0), in_=ps0)
    nc.vector.tensor_copy(out=kT_out(0, 1), in_=ps1)

    # Phase 2a-local RIGHT AFTER the gated block: qT[e, j_local] for this
    # core's OWN 1024 queries, so the AllGather chain launches ~50us in.
    # Its first groups gate on the wq chunk stream (landing ~20-26us);
    # interleaved dummies keep the duty high.  Only wq chunk 0 needs a
    # touch: each group's START matmul carries the PSUM-reuse wait, so it
    # must not also wait on a DMA; later chunks' waits ride legally on the
    # non-start matmuls (one wait each).
    touch(wq_sb[:, 0, :])
    for e in range(EC):
        for it in range(NKH // F):
            ps = psmain.tile([P, F], F32, tag="ps")
            for c in range(DC):
                nc.tensor.matmul(
                    ps,
                    lhsT=wq_sb[:, c, e * P : (e + 1) * P],
                    rhs=xTk_sb[:, c, it * F : (it + 1) * F],
                    start=(c == 0),
                    stop=(c == DC - 1),
                )
                if e == 0:
                    dummy()
            qdst = (
                qTloc_bf[:, e, it * F : (it + 1) * F]
                if e < NBF
                else qTloc_f8[:, e - NBF, it * F : (it + 1) * F]
            )
            nc.vector.tensor_copy(out=qdst, in_=ps)
    # Ship the local shard and AllGather across the core pair.  All bounce
    # traffic rides gpsimd's SWDGE queues (3 + 4 output stores = 7 DMAs,
    # at most one per queue — no queue-lap waits).  One DMA per hop so
    # each instruction carries a single sync wait (the collective cannot
    # aggregate multiple input-piece semaphores).
    cc_in_dma = nc.gpsimd.dma_start(out=cc_in[:, :], in_=qTloc_sb[:, :])
    cc = nc.gpsimd.collective_compute(
        "AllGather",
        mybir.AluOpType.bypass,
        replica_groups=REPLICA_GROUPS,
        ins=[cc_in[:, :].opt()],
        outs=[cc_out[:, :, :].opt()],
    )

    # Phase 1a remainder (kT e-blocks 1-7) + wv touch (wv lands ~31us; the
    # PE reaches e=4 ~62us).
    def kT_block(e):
        for jt in range(NKH // F):
            ps = psmain.tile([P, F], F32, tag="ps")
            for c in range(DC):
                nc.tensor.matmul(
                    ps,
                    lhsT=wk_sb[:, c, e * P : (e + 1) * P],
                    rhs=xTk_sb[:, c, jt * F : (jt + 1) * F],
                    start=(c == 0),
                    stop=(c == DC - 1),
                )
            nc.vector.tensor_copy(out=kT_out(e, jt), in_=ps)

    for e in range(1, EC):
        kT_block(e)
        if e == 4:
            touch(wv_sb[:, 0, :])

    # Phase 1b: v[j, e] — lhsT = xTk[d, j-blk], rhs = WvT[d, e-tile]
    for j in range(JB):
        for et in range(D // F):
            ps = psmain.tile([P, F], F32, tag="ps")
            for c in range(DC):
                nc.tensor.matmul(
                    ps,
                    lhsT=xTk_sb[:, c, j * P : (j + 1) * P],
                    rhs=wv_sb[:, c, et * F : (et + 1) * F],
                    start=(c == 0),
                    stop=(c == DC - 1),
                )
            nc.vector.tensor_copy(out=v_sb[:, j, et * F : (et + 1) * F], in_=ps)

    # Read the gathered qT shards back, one DMA per rank, STAGGERED (the
    # nop serializes rank 1 behind rank 0) so rank 0 gets full read
    # bandwidth and phase 2b can start on its query tiles ~2us sooner.
    # qT_sb is fresh, so each read-back's only dependency is the previous
    # hop — exactly one wait per DMA.
    qt_rb0 = nc.gpsimd.dma_start(out=qT_sb[:, 0, :], in_=cc_out[0])
    n_rb = nc.gpsimd.nop(hint="observe")
    tile.add_dep_helper(n_rb.ins, qt_rb0.ins, reason="stagger rank-1 read-back")
    qt_rb1 = nc.gpsimd.dma_start(out=qT_sb[:, 1, :], in_=cc_out[1])

    # Phase 2b: scoresT[j, i] = k @ q.T over this key half, p = exp(s*SCALE).
    # Mixed contraction: e-blocks 0-3 bf16, e-blocks 4-7 as two fp8
    # DoubleRow matmuls (3D APs [128, 2, n]; middle dim = the interleaved
    # contraction-row pair), accumulating into one PSUM group.  Tiles are
    # processed rank-major (all rank-0 query tiles first) so compute can
    # begin as soon as rank 0's read-back lands; within a rank the
    # bf16/DoubleRow order snakes so consecutive tiles share the PE
    # weight-path mode at the boundary (mode switches cost ~200ns).
    def scores_tile(j, rk, itr, flip):
        q_bf, q_f8 = qT_rk[rk]
        ps = psmain.tile([P, F], F32, tag="ps")
        bf_mms = [
            dict(
                lhsT=kT_bf[:, e, j * P : (j + 1) * P],
                rhs=q_bf[:, e, itr * F : (itr + 1) * F],
                perf_mode=None,
            )
            for e in range(NBF)
        ]
        f8_mms = [
            dict(
                lhsT=kT_f8[:, e : e + 2, j * P : (j + 1) * P],
                rhs=q_f8[:, e : e + 2, itr * F : (itr + 1) * F],
                perf_mode=mybir.MatmulPerfMode.DoubleRow,
            )
            for e in range(0, EC - NBF, 2)
        ]
        mms = bf_mms + f8_mms if not flip else f8_mms + bf_mms
        for i, kw in enumerate(mms):
            nc.tensor.matmul(
                ps,
                lhsT=kw["lhsT"],
                rhs=kw["rhs"],
                start=(i == 0),
                stop=(i == len(mms) - 1),
                perf_mode=kw["perf_mode"],
            )
        return ps

    flip = False
    last_exp = None
    for rk in range(2):
        # Absorb this rank's read-back DMA wait on the PE.
        touch(qT_sb[:, rk, 0:1])
        for j in range(JB):
            for itr in range(NKH // F):
                it = rk * (NKH // F) + itr
                ps = scores_tile(j, rk, itr, flip)
                flip = not flip
                last_exp = nc.scalar.activation(
                    out=pT_sb[:, j, it * F : (it + 1) * F],
                    in_=ps,
                    func=mybir.ActivationFunctionType.Exp,
                    scale=float(SCALE),
                )

    for dmad in in_dmas:
        sp_observe(dmad, "observe input DMA on SP")
    sp_observe(cc_in_dma, "observe cc bounce-in DMA on SP")

    # Phase 2c: partial out[i, 0:1024] = pT.T @ v, partial denom in column
    # 1024 (folded into the same output tensor).  FOUR stores sized
    # {5,1,1,1} query-block groups: the big store issues mid-phase when
    # write bandwidth is free, the three small ones trickle out ~3.6us
    # apart, so the end-of-kernel drain only covers 525KB.  Every store
    # has its own buffer — no WAR guards needed.  Stored bf16.
    outr = out.rearrange("(gg p) e -> p gg e", p=P)   # [P, 16, D+1]
    STORES = [(0, 5), (5, 1), (6, 1), (7, 1)]         # (start ib2, n ib2)
    out_dmas = []
    for s, (start, ng) in enumerate(STORES):
        pool = outp_big if ng > 1 else outp_sm
        o_sb = pool.tile([P, 2 * ng, D + 1], BF, tag="o")
        g2 = nc.vector.memset(o_sb[0:1, 0, 0:1], 0.0)
        for gi in range(ng):
            ib2 = start + gi
            for t in range(2):
                ib = 2 * ib2 + t
                tl = 2 * gi + t
                po0 = psav.tile([P, F], F32, tag="po")
                po1 = psav.tile([P, F], F32, tag="po")
                pd = psav.tile([P, F], F32, tag="po")
                for j in range(JB):
                    lhsT = pT_sb[:, j, ib * P : (ib + 1) * P]
                    nc.tensor.matmul(
                        po0, lhsT=lhsT, rhs=v_sb[:, j, 0:F],
                        start=(j == 0), stop=(j == JB - 1),
                    )
                    nc.tensor.matmul(
                        po1, lhsT=lhsT, rhs=v_sb[:, j, F : 2 * F],
                        start=(j == 0), stop=(j == JB - 1),
                    )
                    last_mm = nc.tensor.matmul(
                        pd[:, 0:1], lhsT=lhsT, rhs=ones_sb,
                        start=(j == 0), stop=(j == JB - 1),
                    )
                # Denominator copy first: pd's stop-matmul is the group's
                # last PE tick, so this copy's PE wait covers po0/po1 and
                # the po copies need only their (buffer-reuse) DVE wait.
                dcp = nc.vector.tensor_copy(
                    out=o_sb[:, tl, D : D + 1], in_=pd[:, 0:1]
                )
                tile.add_dep_helper(
                    dcp.ins, g2.ins, False, reason="order after guard"
                )
                c0 = nc.vector.tensor_copy(out=o_sb[:, tl, 0:F], in_=po0)
                tile.add_dep_helper(c0.ins, dcp.ins, False, reason="order after dcp")
                last_cp = nc.vector.tensor_copy(out=o_sb[:, tl, F : 2 * F], in_=po1)
                tile.add_dep_helper(last_cp.ins, c0.ins, False, reason="order after c0")
        out_dmas.append(
            nc.gpsimd.dma_start(
                out=outr[:, 2 * start : 2 * (start + ng), :], in_=o_sb
            )
        )

    for dd in out_dmas:
        sp_observe(dd, "observe output DMA on SP")
    sp_observe(qt_rb0, "observe qT read-back 0 on SP")
    sp_observe(qt_rb1, "observe qT read-back 1 on SP")
    sp_observe(last_exp, "observe ACT on SP")
    sp_observe(last_mm, "observe PE on SP")
    sp_observe(last_cp, "observe DVE on SP")


def build_attention_module():
    nc = bass.Bass(trn_type="TRN2", target_bir_lowering=False, debug=False)
    xTk = nc.dram_tensor("xTk", [D, NKH], BF, kind="ExternalInput").ap()
    wqT = nc.dram_tensor("wqT", [D, D], BF, kind="ExternalInput").ap()
    wkT = nc.dram_tensor("wkT", [D, D], BF, kind="ExternalInput").ap()
    wvT = nc.dram_tensor("wvT", [D, D], BF, kind="ExternalInput").ap()
    out = nc.dram_tensor("out", [NQ, D + 1], BF, kind="ExternalOutput").ap()
    with tile.TileContext(nc) as tc:
        with ExitStack() as ctx:
            _attention_kernel(ctx, tc, out, xTk, wqT, wkT, wvT)
    return nc


_module_cache = None


def _get_module():
    global _module_cache
    if _module_cache is None:
        _module_cache = build_attention_module()
    return _module_cache


def make_in_maps(x, Wq, Wk, Wv):
    bf = ml_dtypes.bfloat16
    x = np.asarray(x, dtype=np.float32)
    wq = np.asarray(Wq, dtype=np.float32).T.astype(bf)
    wk = np.asarray(Wk, dtype=np.float32).T.astype(bf)
    wv = np.asarray(Wv, dtype=np.float32).T.astype(bf)
    in_maps = []
    for core in range(NCORES):
        b, half = divmod(core, 2)
        xtk = x[b].T[:, half * NKH : (half + 1) * NKH]  # [D, NKH]
        in_maps.append(
            {
                "xTk": np.ascontiguousarray(xtk).astype(bf),
                "wqT": wq,
                "wkT": wk,
                "wvT": wv,
            }
        )
    return in_maps


def _install_ntff_hook_shim():
    """The container's `antenv` stub lacks axon_hooks; register an equivalent
    built on trn_agent_boot's ctypes NTFF driver so trace=True works."""
    import sys
    import types

    if "antenv.axon_hooks" in sys.modules:
        return
    try:
        from trn_agent_boot.trn_boot import _ntff_profile_via_ctypes

        hook = _ntff_profile_via_ctypes("/opt/axon/libaxon_pjrt.so")
    except Exception:
        hook = None
    mod = types.ModuleType("antenv.axon_hooks")
    mod.get_axon_ntff_profile_hook = lambda: hook
    sys.modules["antenv.axon_hooks"] = mod


def kernel(x, Wq, Wk, Wv, _trace=False, _trace_cores=None):
    if _trace:
        _install_ntff_hook_shim()
    in_maps = make_in_maps(x, Wq, Wk, Wv)
    nc = _get_module()
    res = run_bass_kernel_spmd(
        nc,
        in_maps,
        core_ids=list(range(NCORES)),
        trace=_trace,
        trace_cores=_trace_cores,
    )
    out = np.empty((B, N, D), dtype=np.float32)
    for b in range(B):
        r0 = res.results[2 * b]["out"].astype(np.float32)
        r1 = res.results[2 * b + 1]["out"].astype(np.float32)
        osum = r0 + r1
        out[b] = osum[:, :D] / osum[:, D : D + 1]
    if _trace:
        return out, res
    return out



# revision 4
# speedup vs baseline: 1.1731x; 1.1731x over previous
"""Single-head attention (B=4, N=2048, D=1024) on 8 Trainium2 NeuronCores.

Sharding: core c handles batch c//2 and KEY half c%2 (its rows also serve
as its own 1024 queries).

Weight-folding ("M-trick"): scores = q.k^T = x (Wq^T Wk) x^T, so the host
precomputes M = Wq^T Wk once (weights-only preprocessing) and the kernel
computes u = x M for its own queries; the key-side operand of the scores
contraction is then raw x, already resident in SBUF.  This removes the
entire K-projection (65536 PE cycles/core) and the Wk input load.

Pair exchange: instead of AllGather-ing the query shards, the pair runs a
bf16 AllReduce(sum) of u and each core reconstructs the partner's shard
as Z - u_own on the Vector engine.  This keeps the program SPMD-symmetric
while letting every core process its OWN queries first (out rows are
[own | partner]; the host reorders), so the collective hides behind
v-projection + own-half scores/AV (~67us of work).  The exchange is split
into two pipelined AllReduces (e-blocks 0-3 / 4-7) so the first half is
usable ~35us earlier than a monolithic one.

Phase order: gated u e=0 (x/M chunk stream) -> u e=1..7 (AllReduce halves
launched after e=3 / e=7) -> v-projection -> fused scores+AV per 512-query
tile (own tiles, then partner tiles), each tile's output stored as soon as
its AV copies land.

Precision: projections/AV bf16 (fp32 PSUM).  Scores contraction: e-blocks
0-1 bf16, 2-7 fp8e4 DoubleRow (two contraction rows per PE cell).  The
AllReduce runs in bf16 and the subtraction adds ~0.5% noise only to the
partner's u (numpy sim of the full chain: rel err 1.59e-2 vs the 2e-2
gate, identical to the AllGather baseline).  exp in fp32 on the scalar
engine; unnormalized softmax; partial outputs bf16 with the softmax
denominator folded into output column 1024.  Host combines the key-halves:
out = (oA + oB) / (dA + dB).
"""

from contextlib import ExitStack

import ml_dtypes
import numpy as np

import concourse.bass as bass
import concourse.mybir as mybir
import concourse.tile as tile
from concourse.bass_utils import run_bass_kernel_spmd

B, N, D = 4, 2048, 1024
NCORES = 8
P = 128
NQ = N            # total queries per batch
NKH = N // 2      # keys (and own queries) per core
DC = D // P       # 8 contraction chunks
EC = D // P       # 8 embed blocks
JB = NKH // P     # 8 key blocks
F = 512           # matmul moving free dim (one PSUM bank of fp32)
SCALE = 1.0 / np.sqrt(D)
N_WARM_PRE = 5    # dummy matmuls bridging trigger latency until chunk 0
                  # lands; the chunk-gated loop then sustains the HAM
                  # warm-up window
NBF = 2           # scores e-blocks 0..NBF-1 in bf16; the rest fp8 DoubleRow
NF8 = EC - NBF    # 6 fp8 e-blocks
ECH = EC // 2     # e-blocks per AllReduce half

BF = mybir.dt.bfloat16
F8 = mybir.dt.float8e4
F32 = mybir.dt.float32

REPLICA_GROUPS = [[0, 1], [2, 3], [4, 5], [6, 7]]


def _attention_kernel(ctx, tc, out, xTk, mT, wvT):
    nc = tc.nc

    consts = ctx.enter_context(tc.tile_pool(name="consts", bufs=1))
    psmain = ctx.enter_context(tc.tile_pool(name="psmain", bufs=2, space="PSUM"))
    psav = ctx.enter_context(tc.tile_pool(name="psav", bufs=5, space="PSUM"))
    pswarm = ctx.enter_context(tc.tile_pool(name="pswarm", bufs=1, space="PSUM"))
    # One staging buffer per output tile — no reuse, so the per-tile guard
    # memset never carries a WAR wait (DVE memset supports only one).
    outp = ctx.enter_context(tc.tile_pool(name="outp", bufs=4))
    small = ctx.enter_context(tc.tile_pool(name="small", bufs=2))
    dram = ctx.enter_context(tc.tile_pool(name="dram", bufs=1, space="DRAM"))

    # Resident SBUF tensors.
    xTk_sb = consts.tile([P, DC, NKH], BF, tag="xTk")    # [p, d-chunk, key]
    m_sb = consts.tile([P, DC, D], BF, tag="m")          # [p, d-chunk, d']
    wv_sb = consts.tile([P, DC, D], BF, tag="wv")
    uloc_sb = consts.tile([P, EC, NKH], BF, tag="uloc")  # own u, bf16
    uloc_f8 = consts.tile([P, NF8, NKH], F8, tag="ulocf8")
    xk_f8 = consts.tile([P, NF8, NKH], F8, tag="xkf8")   # fp8 x (key side)
    z_sb = consts.tile([P, EC, NKH], BF, tag="z")        # AllReduce result
    upart_bf = consts.tile([P, NBF, NKH], BF, tag="upbf")
    upart_f8 = consts.tile([P, NF8, NKH], F8, tag="upf8")
    v_sb = consts.tile([P, JB, D], BF, tag="v")          # [p, key-block, e]
    pT_sb = consts.tile([P, JB, NQ], BF, tag="pT")       # [p, key-block, query]
    ones_sb = consts.tile([P, 1], BF, tag="ones")

    # DRAM bounce buffers for the two pairwise bf16 AllReduce halves.
    cc_in = [dram.tile([P, ECH * NKH], BF, name=f"cc{h}_in") for h in range(2)]
    cc_out = [dram.tile([P, ECH * NKH], BF, name=f"cc{h}_out") for h in range(2)]

    nc.vector.memset(ones_sb, 1.0)

    xTr = xTk.rearrange("(c p) j -> p c j", p=P)
    mr = mT.rearrange("(c p) e -> p c e", p=P)
    wvr = wvT.rearrange("(c p) e -> p c e", p=P)

    # Input DMAs.  The per-core HBM read port (~358 GB/s) is the early
    # bottleneck: 6MB of input takes ~17us to land.  The gated u e=0 loop
    # needs (x, M) chunk pairs ASAP; wv (needed ~37us in) follows as one
    # large DMA.  Per-queue FIFO on the HWDGE queues preserves priority.
    xm_dmas = []
    for c in range(DC):
        xm_dmas.append(nc.sync.dma_start(out=xTk_sb[:, c, :], in_=xTr[:, c, :]))
        xm_dmas.append(nc.sync.dma_start(out=m_sb[:, c, :], in_=mr[:, c, :]))
    wv_dma = nc.sync.dma_start(out=wv_sb[:, :, :], in_=wvr)
    in_dmas = xm_dmas + [wv_dma]

    def sp_observe(inst, why):
        n = nc.sync.nop(hint="observe")
        tile.add_dep_helper(n.ins, inst.ins, reason=why)

    # Warm/touch PSUM tile in its own bank: nothing ever reads it, so every
    # write is PE-local and touch matmuls carry exactly one (DMA/DVE) wait.
    warm_src = small.tile([P, 640], BF, tag="warm")
    nc.vector.memset(warm_src, 0.0)
    warm_ps = pswarm.tile([P, F], F32, tag="wps")

    def dummy():
        nc.tensor.matmul(
            warm_ps, lhsT=warm_src[:, 0:P], rhs=warm_src[:, P : P + F],
            start=True, stop=True,
        )

    def touch(t):
        # Trivial matmul whose only purpose is to make the PE observe t's
        # producer (single sync wait), so later real matmuls need none.
        nc.tensor.matmul(
            warm_ps[0:1, 0:1], lhsT=t[:, 0:1], rhs=t[:, 0:1], start=True, stop=True
        )

    # Solid warm-up block: HAM un-throttles only after a ~3.4us window of
    # SUSTAINED PE activity.
    for _ in range(N_WARM_PRE):
        dummy()

    # Phase 1 e=0: uT[0, it] — chunk-major, gated on each (x, M) chunk pair
    # as it lands (~1.4us apart at HBM rate), touch matmuls carrying the
    # DMA waits and interleaved dummies keeping the PE duty high.
    ps0 = psmain.tile([P, F], F32, tag="ps")
    ps1 = psmain.tile([P, F], F32, tag="ps")
    for c in range(DC):
        touch(xTk_sb[:, c, :])
        touch(m_sb[:, c, :])
        nc.tensor.matmul(
            ps0, lhsT=m_sb[:, c, 0:P], rhs=xTk_sb[:, c, 0:F],
            start=(c == 0), stop=(c == DC - 1),
        )
        nc.tensor.matmul(
            ps1, lhsT=m_sb[:, c, 0:P], rhs=xTk_sb[:, c, F : 2 * F],
            start=(c == 0), stop=(c == DC - 1),
        )
        dummy()
        dummy()
    nc.vector.tensor_copy(out=uloc_sb[:, 0, 0:F], in_=ps0)
    nc.vector.tensor_copy(out=uloc_sb[:, 0, F : 2 * F], in_=ps1)

    # Phase 1 remainder (u e-blocks 1-7); AllReduce half h launches as soon
    # as its 4 e-blocks are copied, so exchange overlaps the rest of the
    # projections and the own-half scores/AV.
    cc_in_dmas = []
    ccs = []

    def launch_cc(h):
        d = nc.gpsimd.dma_start(
            out=cc_in[h][:, :], in_=uloc_sb[:, h * ECH : (h + 1) * ECH, :]
        )
        cc = nc.gpsimd.collective_compute(
            "AllReduce",
            mybir.AluOpType.add,
            replica_groups=REPLICA_GROUPS,
            ins=[cc_in[h][:, :].opt()],
            outs=[cc_out[h][:, :].opt()],
        )
        cc_in_dmas.append(d)
        ccs.append(cc)

    for e in range(1, EC):
        for it in range(NKH // F):
            ps = psmain.tile([P, F], F32, tag="ps")
            for c in range(DC):
                nc.tensor.matmul(
                    ps,
                    lhsT=m_sb[:, c, e * P : (e + 1) * P],
                    rhs=xTk_sb[:, c, it * F : (it + 1) * F],
                    start=(c == 0),
                    stop=(c == DC - 1),
                )
            nc.vector.tensor_copy(
                out=uloc_sb[:, e, it * F : (it + 1) * F], in_=ps
            )
        if e == ECH - 1:
            launch_cc(0)
    launch_cc(1)

    # fp8 casts for the scores contraction (DVE, off the critical path):
    # own u e-blocks 2-7 and the key-side x e-blocks 2-7.  The x cast is
    # per-block: each block depends on a different input-DMA queue
    # semaphore, and one instruction can only carry a few sync waits.
    nc.vector.tensor_copy(out=uloc_f8[:, :, :], in_=uloc_sb[:, NBF:EC, :])
    for e in range(NF8):
        nc.vector.tensor_copy(out=xk_f8[:, e, :], in_=xTk_sb[:, NBF + e, :])

    # Phase 2: v[j, e] — lhsT = xTk[d, j-blk], rhs = WvT[d, e-tile]
    touch(wv_sb[:, 0, :])
    for j in range(JB):
        for et in range(D // F):
            ps = psmain.tile([P, F], F32, tag="ps")
            for c in range(DC):
                nc.tensor.matmul(
                    ps,
                    lhsT=xTk_sb[:, c, j * P : (j + 1) * P],
                    rhs=wv_sb[:, c, et * F : (et + 1) * F],
                    start=(c == 0),
                    stop=(c == DC - 1),
                )
            nc.vector.tensor_copy(out=v_sb[:, j, et * F : (et + 1) * F], in_=ps)

    # Read the AllReduce halves back, STAGGERED (the nop serializes hop 1
    # behind hop 0) so hop 0 gets full read bandwidth.  z_sb is fresh, so
    # each read-back's only dependency is its collective — one wait each.
    rb0 = nc.gpsimd.dma_start(out=z_sb[:, 0:ECH, :], in_=cc_out[0][:, :])
    n_rb = nc.gpsimd.nop(hint="observe")
    tile.add_dep_helper(n_rb.ins, rb0.ins, reason="stagger read-back 1")
    rb1 = nc.gpsimd.dma_start(out=z_sb[:, ECH:EC, :], in_=cc_out[1][:, :])

    # Partner reconstruction on DVE: upart = Z - u_own.  Blocks 0-1 stay
    # bf16 (scores bf16 operand), 2-7 cast straight to fp8.
    nc.vector.tensor_sub(
        upart_bf[:, :, :], z_sb[:, 0:NBF, :], uloc_sb[:, 0:NBF, :]
    )
    nc.vector.tensor_sub(
        upart_f8[:, 0 : ECH - NBF, :], z_sb[:, NBF:ECH, :], uloc_sb[:, NBF:ECH, :]
    )
    nc.vector.tensor_sub(
        upart_f8[:, ECH - NBF : NF8, :], z_sb[:, ECH:EC, :], uloc_sb[:, ECH:EC, :]
    )

    # Phase 3: fused scores+AV per 512-query tile.  Tiles 0-1 = OWN queries
    # (u from uloc, no exchange dependency), tiles 2-3 = partner queries.
    # scoresT[j, i] = sum_d' xT[d', j] uT[d', i]; p = exp(s*SCALE); then
    # out[i, 0:1024] = pT.T @ v with the denominator folded into column
    # 1024.  Within a tile the bf16/DoubleRow order snakes per j so
    # consecutive groups share the PE weight-path mode at the boundary.
    outr = out.rearrange("(gg p) e -> p gg e", p=P)   # [P, 16, D+1]
    out_dmas = []
    flip = False
    last_exp = last_mm = last_cp = None

    for t in range(2 * (NKH // F)):
        own = t < NKH // F
        itc = t if own else t - NKH // F
        u_bf = uloc_sb if own else None
        if t == NKH // F:
            # Absorb the read-back/subtract waits on the PE so the AV
            # START matmuls never carry a second wait.
            touch(upart_bf[:, 0, 0:1])
            touch(upart_f8[:, 0, 0:1])
            touch(upart_f8[:, ECH - NBF, 0:1])
        for j in range(JB):
            ps = psmain.tile([P, F], F32, tag="ps")
            bf_mms = [
                dict(
                    lhsT=xTk_sb[:, e, j * P : (j + 1) * P],
                    rhs=(uloc_sb[:, e, itc * F : (itc + 1) * F] if own
                         else upart_bf[:, e, itc * F : (itc + 1) * F]),
                    perf_mode=None,
                )
                for e in range(NBF)
            ]
            uf8 = uloc_f8 if own else upart_f8
            f8_mms = [
                dict(
                    lhsT=xk_f8[:, e : e + 2, j * P : (j + 1) * P],
                    rhs=uf8[:, e : e + 2, itc * F : (itc + 1) * F],
                    perf_mode=mybir.MatmulPerfMode.DoubleRow,
                )
                for e in range(0, NF8, 2)
            ]
            mms = bf_mms + f8_mms if not flip else f8_mms + bf_mms
            for i, kw in enumerate(mms):
                nc.tensor.matmul(
                    ps,
                    lhsT=kw["lhsT"],
                    rhs=kw["rhs"],
                    start=(i == 0),
                    stop=(i == len(mms) - 1),
                    perf_mode=kw["perf_mode"],
                )
            flip = not flip
            last_exp = nc.scalar.activation(
                out=pT_sb[:, j, t * F : (t + 1) * F],
                in_=ps,
                func=mybir.ActivationFunctionType.Exp,
                scale=float(SCALE),
            )
        # AV for this tile's 4 query blocks; store as soon as copies land.
        o_sb = outp.tile([P, 4, D + 1], BF, tag="o")
        g2 = nc.vector.memset(o_sb[0:1, 0, 0:1], 0.0)
        for g in range(4):
            ib = 4 * t + g
            po0 = psav.tile([P, F], F32, tag="po")
            po1 = psav.tile([P, F], F32, tag="po")
            pd = psav.tile([P, F], F32, tag="po")
            for j in range(JB):
                lhsT = pT_sb[:, j, ib * P : (ib + 1) * P]
                nc.tensor.matmul(
                    po0, lhsT=lhsT, rhs=v_sb[:, j, 0:F],
                    start=(j == 0), stop=(j == JB - 1),
                )
                nc.tensor.matmul(
                    po1, lhsT=lhsT, rhs=v_sb[:, j, F : 2 * F],
                    start=(j == 0), stop=(j == JB - 1),
                )
                last_mm = nc.tensor.matmul(
                    pd[:, 0:1], lhsT=lhsT, rhs=ones_sb,
                    start=(j == 0), stop=(j == JB - 1),
                )
            # Denominator copy first: pd's stop-matmul is the group's last
            # PE tick, so this copy's PE wait covers po0/po1 and the po
            # copies need only their (buffer-reuse) DVE wait.
            dcp = nc.vector.tensor_copy(out=o_sb[:, g, D : D + 1], in_=pd[:, 0:1])
            tile.add_dep_helper(dcp.ins, g2.ins, False, reason="order after guard")
            c0 = nc.vector.tensor_copy(out=o_sb[:, g, 0:F], in_=po0)
            tile.add_dep_helper(c0.ins, dcp.ins, False, reason="order after dcp")
            last_cp = nc.vector.tensor_copy(out=o_sb[:, g, F : 2 * F], in_=po1)
            tile.add_dep_helper(last_cp.ins, c0.ins, False, reason="order after c0")
        out_dmas.append(
            nc.gpsimd.dma_start(out=outr[:, 4 * t : 4 * (t + 1), :], in_=o_sb)
        )

    for dmad in in_dmas:
        sp_observe(dmad, "observe input DMA on SP")
    for d in cc_in_dmas:
        sp_observe(d, "observe cc bounce-in DMA on SP")
    for dd in out_dmas:
        sp_observe(dd, "observe output DMA on SP")
    sp_observe(rb0, "observe read-back 0 on SP")
    sp_observe(rb1, "observe read-back 1 on SP")
    sp_observe(last_exp, "observe ACT on SP")
    sp_observe(last_mm, "observe PE on SP")
    sp_observe(last_cp, "observe DVE on SP")


def build_attention_module():
    nc = bass.Bass(trn_type="TRN2", target_bir_lowering=False, debug=False)
    xTk = nc.dram_tensor("xTk", [D, NKH], BF, kind="ExternalInput").ap()
    mT = nc.dram_tensor("mT", [D, D], BF, kind="ExternalInput").ap()
    wvT = nc.dram_tensor("wvT", [D, D], BF, kind="ExternalInput").ap()
    out = nc.dram_tensor("out", [NQ, D + 1], BF, kind="ExternalOutput").ap()
    with tile.TileContext(nc) as tc:
        with ExitStack() as ctx:
            _attention_kernel(ctx, tc, out, xTk, mT, wvT)
    return nc


_module_cache = None


def _get_module():
    global _module_cache
    if _module_cache is None:
        _module_cache = build_attention_module()
    return _module_cache


def make_in_maps(x, Wq, Wk, Wv):
    bf = ml_dtypes.bfloat16
    x = np.asarray(x, dtype=np.float32)
    wq = np.asarray(Wq, dtype=np.float32)
    wk = np.asarray(Wk, dtype=np.float32)
    m = (wq.T @ wk).astype(bf)                       # [d, d'], contraction-first
    wv = np.asarray(Wv, dtype=np.float32).T.astype(bf)
    in_maps = []
    for core in range(NCORES):
        b, half = divmod(core, 2)
        xtk = x[b].T[:, half * NKH : (half + 1) * NKH]  # [D, NKH]
        in_maps.append(
            {
                "xTk": np.ascontiguousarray(xtk).astype(bf),
                "mT": m,
                "wvT": wv,
            }
        )
    return in_maps


def _install_ntff_hook_shim():
    """The container's `antenv` stub lacks axon_hooks; register an equivalent
    built on trn_agent_boot's ctypes NTFF driver so trace=True works."""
    import sys
    import types

    if "antenv.axon_hooks" in sys.modules:
        return
    try:
        from trn_agent_boot.trn_boot import _ntff_profile_via_ctypes

        hook = _ntff_profile_via_ctypes("/opt/axon/libaxon_pjrt.so")
    except Exception:
        hook = None
    mod = types.ModuleType("antenv.axon_hooks")
    mod.get_axon_ntff_profile_hook = lambda: hook
    sys.modules["antenv.axon_hooks"] = mod


def kernel(x, Wq, Wk, Wv, _trace=False, _trace_cores=None):
    if _trace:
        _install_ntff_hook_shim()
    in_maps = make_in_maps(x, Wq, Wk, Wv)
    nc = _get_module()
    res = run_bass_kernel_spmd(
        nc,
        in_maps,
        core_ids=list(range(NCORES)),
        trace=_trace,
        trace_cores=_trace_cores,
    )
    out = np.empty((B, N, D), dtype=np.float32)
    for b in range(B):
        # Core rows are [own-half | partner-half]: half-0 cores are already
        # in global query order; half-1 cores need their halves swapped.
        r0 = res.results[2 * b]["out"].astype(np.float32)
        r1 = res.results[2 * b + 1]["out"].astype(np.float32)
        r1 = np.concatenate([r1[NKH:], r1[:NKH]], axis=0)
        osum = r0 + r1
        out[b] = osum[:, :D] / osum[:, D : D + 1]
    if _trace:
        return out, res
    return out


# revision 43
# speedup vs baseline: 1.1877x; 1.0125x over previous
"""Single-head attention (B=4, N=2048, D=1024) on 8 Trainium2 NeuronCores.

Sharding: core c handles batch c//2 and KEY half c%2 (its rows also serve
as its own 1024 queries).

Weight-folding ("M-trick"): scores = q.k^T = x (Wq^T Wk) x^T, so the host
precomputes M = Wq^T Wk once (weights-only preprocessing) and the kernel
computes u = x M for its own queries; the key-side operand of the scores
contraction is then raw x, already resident in SBUF.  This removes the
entire K-projection (65536 PE cycles/core) and the Wk input load.

Pair exchange: instead of AllGather-ing the query shards, the pair runs a
bf16 AllReduce(sum) of u and each core reconstructs the partner's shard
as Z - u_own on the Vector engine.  This keeps the program SPMD-symmetric
while letting every core process its OWN queries first (out rows are
[own | partner]; the host reorders), so the collective hides behind
v-projection + own-half scores/AV.  The exchange is split into two
pipelined AllReduces (e-blocks 0-3 / 4-7); measured: mesh begins ~18us
after launch, each 1MB mesh takes ~21-26us, and the halves pipeline
back-to-back on the CC engine.

Inputs stream as four 1MB pieces (x split by key-half, M split by
e-block-half) + wv, in that trigger order: HWDGE aggregate is only ~190
GB/s and per-queue FIFO drains pieces in trigger order, so u(e0-3, it0)
starts when just (xa, ma) have landed (~14us) instead of after all of
x+M (~24us).  This pulls u-proj AND the collective chain ~10us left.

Precision: projections/AV bf16 (fp32 PSUM).  Scores contraction: ALL 8
e-blocks fp8e4 DoubleRow (a DR matmul costs the same 216ns as bf16 per
512-free tile, so the win is 4 instead of 8 matmuls per tile).  The
AllReduce runs in bf16; the subtraction adds ~0.5% noise only to the
partner's u.  numpy sim of the full chain (bit-exact vs hardware at
NBF=2) predicts rel err 1.81e-2; hardware measures 1.83e-2 vs the 2e-2
gate.  exp in fp32 on the scalar engine; unnormalized softmax; partial
outputs bf16 with the softmax denominator folded into output column
1024.  Host combines the key-halves: out = (oA + oB) / (dA + dB).
"""

from contextlib import ExitStack

import ml_dtypes
import numpy as np

import concourse.bass as bass
import concourse.mybir as mybir
import concourse.tile as tile
from concourse.bass_utils import run_bass_kernel_spmd

B, N, D = 4, 2048, 1024
NCORES = 8
P = 128
NQ = N            # total queries per batch
NKH = N // 2      # keys (and own queries) per core
DC = D // P       # 8 contraction chunks
EC = D // P       # 8 embed blocks
JB = NKH // P     # 8 key blocks
F = 512           # matmul moving free dim (one PSUM bank of fp32)
SCALE = 1.0 / np.sqrt(D)
N_WARM_PRE = 26   # dummy matmuls bridging trigger latency + (xa, ma) load
                  # (~14us) while keeping the HAM warm-up window sustained
ECH = EC // 2     # e-blocks per AllReduce half

BF = mybir.dt.bfloat16
F8 = mybir.dt.float8e4
F32 = mybir.dt.float32

REPLICA_GROUPS = [[0, 1], [2, 3], [4, 5], [6, 7]]


def _attention_kernel(ctx, tc, out, xTa, xTb, mTa, mTb, wvT):
    nc = tc.nc

    consts = ctx.enter_context(tc.tile_pool(name="consts", bufs=1))
    psmain = ctx.enter_context(tc.tile_pool(name="psmain", bufs=2, space="PSUM"))
    psav = ctx.enter_context(tc.tile_pool(name="psav", bufs=5, space="PSUM"))
    pswarm = ctx.enter_context(tc.tile_pool(name="pswarm", bufs=1, space="PSUM"))
    # One staging buffer per output store — no reuse, so the per-store guard
    # memset never carries a WAR wait (DVE memset supports only one).
    outp8 = ctx.enter_context(tc.tile_pool(name="outp8", bufs=1))
    outp4 = ctx.enter_context(tc.tile_pool(name="outp4", bufs=1))
    outp3 = ctx.enter_context(tc.tile_pool(name="outp3", bufs=1))
    outp1 = ctx.enter_context(tc.tile_pool(name="outp1", bufs=1))
    small = ctx.enter_context(tc.tile_pool(name="small", bufs=2))
    dram = ctx.enter_context(tc.tile_pool(name="dram", bufs=1, space="DRAM"))

    # Resident SBUF tensors.  x and M are half-major so each half's DMA
    # lands in one contiguous 8KB-per-partition block (big descriptors):
    # x by KEY half (xTk_sb[:, h, c, k]), M by E-BLOCK half.
    xTk_sb = consts.tile([P, 2, DC, F], BF, tag="xTk")
    m_sb = consts.tile([P, 2, DC, F], BF, tag="m")
    wv_sb = consts.tile([P, DC, D], BF, tag="wv")
    uloc_sb = consts.tile([P, EC, NKH], BF, tag="uloc")  # own u, bf16
    uloc_f8 = consts.tile([P, EC, NKH], F8, tag="ulocf8")
    xk_f8 = consts.tile([P, 2, EC, F], F8, tag="xkf8")   # fp8 x (key side)
    z_sb = consts.tile([P, EC, NKH], BF, tag="z")        # AllReduce result
    upart_f8 = consts.tile([P, EC, NKH], F8, tag="upf8")
    v_sb = consts.tile([P, JB, D], BF, tag="v")          # [p, key-block, e]
    pT_sb = consts.tile([P, JB, NQ], BF, tag="pT")       # [p, key-block, query]
    ones_sb = consts.tile([P, 1], BF, tag="ones")

    def xk_bf(c, j):
        # key-side bf16 x slice [128, 128] for key block j, chunk c
        return xTk_sb[:, j // 4, c, (j % 4) * P : (j % 4 + 1) * P]

    def m_lhsT(c, e):
        return m_sb[:, e // 4, c, (e % 4) * P : (e % 4 + 1) * P]

    # DRAM bounce buffers for the two pairwise bf16 AllReduce halves.
    cc_in = [dram.tile([P, ECH * NKH], BF, name=f"cc{h}_in") for h in range(2)]
    cc_out = [dram.tile([P, ECH * NKH], BF, name=f"cc{h}_out") for h in range(2)]

    nc.vector.memset(ones_sb, 1.0)

    # Input DMAs: four 1MB pieces + wv, drained in trigger order by the
    # per-queue FIFOs.  (xa, ma) unlock u(e0-3, it0); xb unlocks it1; mb
    # unlocks e4-7; wv is only needed by v-projection (~42us).
    in_dmas = []
    dma_xa = nc.sync.dma_start(out=xTk_sb[:, 0, :, :], in_=xTa[:, :])
    dma_ma = nc.sync.dma_start(out=m_sb[:, 0, :, :], in_=mTa[:, :])
    dma_xb = nc.sync.dma_start(out=xTk_sb[:, 1, :, :], in_=xTb[:, :])
    dma_mb = nc.sync.dma_start(out=m_sb[:, 1, :, :], in_=mTb[:, :])
    wv_dma = nc.sync.dma_start(out=wv_sb[:, :, :], in_=wvT[:, :])
    in_dmas += [dma_xa, dma_ma, dma_xb, dma_mb, wv_dma]

    def sp_observe(inst, why):
        n = nc.sync.nop(hint="observe")
        tile.add_dep_helper(n.ins, inst.ins, reason=why)

    # Warm/touch PSUM tile in its own bank: nothing ever reads it, so every
    # write is PE-local and touch matmuls carry exactly one (DMA/DVE) wait.
    warm_src = small.tile([P, 640], BF, tag="warm")
    nc.vector.memset(warm_src, 0.0)
    warm_ps = pswarm.tile([P, F], F32, tag="wps")

    def dummy():
        nc.tensor.matmul(
            warm_ps, lhsT=warm_src[:, 0:P], rhs=warm_src[:, P : P + F],
            start=True, stop=True,
        )

    def touch(t):
        # Trivial matmul whose only purpose is to make the PE observe t's
        # producer (single sync wait), so later real matmuls need none.
        return nc.tensor.matmul(
            warm_ps[0:1, 0:1], lhsT=t[:, 0:1], rhs=t[:, 0:1], start=True, stop=True
        )

    # Solid warm-up block bridging until (xa, ma) land (~14us): HAM
    # un-throttles only after a ~3.4us window of SUSTAINED PE activity.
    for _ in range(N_WARM_PRE):
        dummy()
    touch(xTk_sb[:, 0, 0, :])
    touch(m_sb[:, 0, 0, :])

    # Phase 1: uT[e, it] = sum_c M[c, e-blk]^T x[c, it-tile], piece-gated:
    # e0-3 x it0 first (xa+ma), then it1 (xb), then e4-7 (mb).  AllReduce
    # half h launches as soon as its 4 e-blocks are copied.
    cc_in_dmas = []
    ccs = []

    def launch_cc(h):
        d = nc.gpsimd.dma_start(
            out=cc_in[h][:, :], in_=uloc_sb[:, h * ECH : (h + 1) * ECH, :]
        )
        cc = nc.gpsimd.collective_compute(
            "AllReduce",
            mybir.AluOpType.add,
            replica_groups=REPLICA_GROUPS,
            ins=[cc_in[h][:, :].opt()],
            outs=[cc_out[h][:, :].opt()],
        )
        cc_in_dmas.append(d)
        ccs.append(cc)

    def u_tile(e, it):
        ps = psmain.tile([P, F], F32, tag="ps")
        for c in range(DC):
            nc.tensor.matmul(
                ps,
                lhsT=m_lhsT(c, e),
                rhs=xTk_sb[:, it, c, :],
                start=(c == 0),
                stop=(c == DC - 1),
            )
        nc.vector.tensor_copy(out=uloc_sb[:, e, it * F : (it + 1) * F], in_=ps)

    for e in range(ECH):
        u_tile(e, 0)
    touch(xTk_sb[:, 1, 0, :])
    for e in range(ECH):
        u_tile(e, 1)
    launch_cc(0)
    touch(m_sb[:, 1, 0, :])
    for e in range(ECH, EC):
        u_tile(e, 0)
        u_tile(e, 1)
    launch_cc(1)

    # Phase 2: v[j, e] — lhsT = xTk[d, j-blk], rhs = WvT[d, e-tile].
    # The fp8 casts for the scores contraction (x and own-u pieces) are
    # interleaved between the v copies: a monolithic cast placed before
    # v-proj delays its PSUM-reuse waits (measured 2.2us PE stall), and
    # placed after it would stall tile-0's scores.  Tiny touch copies
    # first so the cast pieces carry no DMA waits.
    dve_scratch = small.tile([P, 2], BF, tag="dvescratch")
    for h in range(2):
        nc.vector.tensor_copy(
            out=dve_scratch[0:1, h : h + 1], in_=xTk_sb[0:1, h, 0, 0:1]
        )
    touch(wv_sb[:, 0, :])
    pieces = [("x", h, e) for h in range(2) for e in range(EC)]
    pieces += [("u", 0, e) for e in range(EC)]
    npc = len(pieces)
    pi = 0
    for j in range(JB):
        for et in range(D // F):
            ps = psmain.tile([P, F], F32, tag="ps")
            for c in range(DC):
                nc.tensor.matmul(
                    ps,
                    lhsT=xk_bf(c, j),
                    rhs=wv_sb[:, c, et * F : (et + 1) * F],
                    start=(c == 0),
                    stop=(c == DC - 1),
                )
            nc.vector.tensor_copy(out=v_sb[:, j, et * F : (et + 1) * F], in_=ps)
            for _ in range((npc - pi) // (16 - (2 * j + et))):
                kind, h, e = pieces[pi]
                pi += 1
                if kind == "x":
                    nc.vector.tensor_copy(
                        out=xk_f8[:, h, e, :], in_=xTk_sb[:, h, e, :]
                    )
                else:
                    nc.vector.tensor_copy(
                        out=uloc_f8[:, e, :], in_=uloc_sb[:, e, :]
                    )

    # Read the AllReduce halves back, STAGGERED (the nop serializes hop 1
    # behind hop 0) so hop 0 gets full read bandwidth.  z_sb is fresh, so
    # each read-back's only dependency is its collective — one wait each.
    rb0 = nc.gpsimd.dma_start(out=z_sb[:, 0:ECH, :], in_=cc_out[0][:, :])
    n_rb = nc.gpsimd.nop(hint="observe")
    tile.add_dep_helper(n_rb.ins, rb0.ins, reason="stagger read-back 1")
    rb1 = nc.gpsimd.dma_start(out=z_sb[:, ECH:EC, :], in_=cc_out[1][:, :])

    # Partner reconstruction on DVE: upart = Z - u_own, cast straight to
    # fp8 (DVE computes in fp32 internally).
    sub0 = nc.vector.tensor_sub(
        upart_f8[:, 0:ECH, :], z_sb[:, 0:ECH, :], uloc_sb[:, 0:ECH, :]
    )
    sub1 = nc.vector.tensor_sub(
        upart_f8[:, ECH:EC, :], z_sb[:, ECH:EC, :], uloc_sb[:, ECH:EC, :]
    )

    # Phase 3: fused scores+AV per 512-query tile.  Tiles 0-1 = OWN queries
    # (u from uloc, no exchange dependency), tiles 2-3 = partner queries.
    # scoresT[j, i] = sum_d' xT[d', j] uT[d', i] — all-fp8 DoubleRow, 4
    # matmuls per (j, tile); p = exp(s*SCALE); then out[i, 0:1024] =
    # pT.T @ v with the denominator folded into column 1024.
    outr = out.rearrange("(gg p) e -> p gg e", p=P)   # [P, 16, D+1]
    STORE_AT = {0: 8, 8: 4, 12: 3, 15: 1}             # start ib -> n blocks
    out_dmas = []
    last_exp = last_mm = last_cp = None

    for t in range(2 * (NKH // F)):
        own = t < NKH // F
        itc = t if own else t - NKH // F
        uf8 = uloc_f8 if own else upart_f8
        if t == NKH // F:
            # Absorb the read-back/subtract waits on the PE so the AV
            # START matmuls never carry a second wait.  Order-pinned after
            # tile 1's last AV matmul: without the pin the scheduler hoists
            # these waits ahead of tile 0/1 and stalls the PE on the
            # exchange (measured 6.6us).
            tch0 = touch(upart_f8[:, 0, 0:1])
            tile.add_dep_helper(
                tch0.ins, last_mm.ins, False, reason="pin after tile1 AV"
            )
            tch1 = touch(upart_f8[:, ECH, 0:1])
            tile.add_dep_helper(
                tch1.ins, tch0.ins, False, reason="pin after first touch"
            )
        for j in range(JB):
            ps = psmain.tile([P, F], F32, tag="ps")
            for i, e in enumerate(range(0, EC, 2)):
                nc.tensor.matmul(
                    ps,
                    lhsT=xk_f8[:, j // 4, e : e + 2, (j % 4) * P : (j % 4 + 1) * P],
                    rhs=uf8[:, e : e + 2, itc * F : (itc + 1) * F],
                    start=(i == 0),
                    stop=(i == EC // 2 - 1),
                    perf_mode=mybir.MatmulPerfMode.DoubleRow,
                )
            last_exp = nc.scalar.activation(
                out=pT_sb[:, j, t * F : (t + 1) * F],
                in_=ps,
                func=mybir.ActivationFunctionType.Exp,
                scale=float(SCALE),
            )
        # AV for this tile's 4 query blocks.  Stores are balanced as
        # [t0+t1: 8 blocks][t2: 4][t3: 3][t3: 1] — gpsimd's SWDGE ring
        # supports only ~8 wait-free DMAs total (4 exchange hops + 4
        # stores), and this split leaves a 262KB final drain.
        for g in range(4):
            ib = 4 * t + g
            if ib in STORE_AT:
                ngrp = STORE_AT[ib]
                pool = {8: outp8, 4: outp4, 3: outp3, 1: outp1}[ngrp]
                o_sb = pool.tile([P, ngrp, D + 1], BF, tag="o")
                o_sb_base = ib
                g2 = nc.vector.memset(o_sb[0:1, 0, 0:1], 0.0)
            gl = ib - o_sb_base
            po0 = psav.tile([P, F], F32, tag="po")
            po1 = psav.tile([P, F], F32, tag="po")
            pd = psav.tile([P, F], F32, tag="po")
            for j in range(JB):
                lhsT = pT_sb[:, j, ib * P : (ib + 1) * P]
                nc.tensor.matmul(
                    po0, lhsT=lhsT, rhs=v_sb[:, j, 0:F],
                    start=(j == 0), stop=(j == JB - 1),
                )
                nc.tensor.matmul(
                    po1, lhsT=lhsT, rhs=v_sb[:, j, F : 2 * F],
                    start=(j == 0), stop=(j == JB - 1),
                )
                last_mm = nc.tensor.matmul(
                    pd[:, 0:1], lhsT=lhsT, rhs=ones_sb,
                    start=(j == 0), stop=(j == JB - 1),
                )
            # Denominator copy first: pd's stop-matmul is the group's last
            # PE tick, so this copy's PE wait covers po0/po1 and the po
            # copies need only their (buffer-reuse) DVE wait.
            dcp = nc.vector.tensor_copy(out=o_sb[:, gl, D : D + 1], in_=pd[:, 0:1])
            tile.add_dep_helper(dcp.ins, g2.ins, False, reason="order after guard")
            if ib == 3:
                # Keep sub0/sub1 ahead of the tile's last copy group in the
                # DVE queue: left to itself the scheduler parks them after
                # ALL tile copies and tile 2 stalls ~5us on the exchange.
                tile.add_dep_helper(dcp.ins, sub0.ins, False, reason="sub0 first")
            if ib == 7:
                tile.add_dep_helper(dcp.ins, sub1.ins, False, reason="sub1 first")
            c0 = nc.vector.tensor_copy(out=o_sb[:, gl, 0:F], in_=po0)
            tile.add_dep_helper(c0.ins, dcp.ins, False, reason="order after dcp")
            last_cp = nc.vector.tensor_copy(out=o_sb[:, gl, F : 2 * F], in_=po1)
            tile.add_dep_helper(last_cp.ins, c0.ins, False, reason="order after c0")
            if gl == ngrp - 1:
                # gpsimd SWDGE only: the HWDGE queues already ran the input
                # DMAs, and a reused queue adds a ring-lap wait on top of
                # the store's DVE data dep (one wait per instruction).
                out_dmas.append(
                    nc.gpsimd.dma_start(
                        out=outr[:, o_sb_base : o_sb_base + ngrp, :], in_=o_sb
                    )
                )

    for dmad in in_dmas:
        sp_observe(dmad, "observe input DMA on SP")
    for d in cc_in_dmas:
        sp_observe(d, "observe cc bounce-in DMA on SP")
    for dd in out_dmas:
        sp_observe(dd, "observe output DMA on SP")
    sp_observe(rb0, "observe read-back 0 on SP")
    sp_observe(rb1, "observe read-back 1 on SP")
    sp_observe(last_exp, "observe ACT on SP")
    sp_observe(last_mm, "observe PE on SP")
    sp_observe(last_cp, "observe DVE on SP")


def build_attention_module():
    nc = bass.Bass(trn_type="TRN2", target_bir_lowering=False, debug=False)
    xTa = nc.dram_tensor("xTa", [P, DC * F], BF, kind="ExternalInput").ap()
    xTb = nc.dram_tensor("xTb", [P, DC * F], BF, kind="ExternalInput").ap()
    mTa = nc.dram_tensor("mTa", [P, DC * F], BF, kind="ExternalInput").ap()
    mTb = nc.dram_tensor("mTb", [P, DC * F], BF, kind="ExternalInput").ap()
    wvT = nc.dram_tensor("wvT", [P, DC * D], BF, kind="ExternalInput").ap()
    out = nc.dram_tensor("out", [NQ, D + 1], BF, kind="ExternalOutput").ap()
    with tile.TileContext(nc) as tc:
        with ExitStack() as ctx:
            _attention_kernel(ctx, tc, out, xTa, xTb, mTa, mTb, wvT)
    return nc


_module_cache = None


def _get_module():
    global _module_cache
    if _module_cache is None:
        _module_cache = build_attention_module()
    return _module_cache


def _chunk_blocked(a):
    """[D, n] -> [128, 8*n]: partition p's row = [chunk0 | chunk1 | ...],
    one contiguous DRAM row per partition (big DMA descriptors)."""
    d, n = a.shape
    return np.ascontiguousarray(
        a.reshape(DC, P, n).transpose(1, 0, 2).reshape(P, DC * n)
    )


def _col_halves(a):
    """[D, 1024] -> two [128, 8*512] chunk-blocked column halves."""
    r = a.reshape(DC, P, 2 * F)
    return (
        np.ascontiguousarray(r[:, :, 0:F].transpose(1, 0, 2).reshape(P, DC * F)),
        np.ascontiguousarray(r[:, :, F : 2 * F].transpose(1, 0, 2).reshape(P, DC * F)),
    )


def make_in_maps(x, Wq, Wk, Wv):
    bf = ml_dtypes.bfloat16
    x = np.asarray(x, dtype=np.float32)
    wq = np.asarray(Wq, dtype=np.float32)
    wk = np.asarray(Wk, dtype=np.float32)
    ma, mb = _col_halves((wq.T @ wk).astype(bf))          # contraction-first
    wv = _chunk_blocked(np.asarray(Wv, dtype=np.float32).T.astype(bf))
    in_maps = []
    for core in range(NCORES):
        b, half = divmod(core, 2)
        xtk = x[b].T[:, half * NKH : (half + 1) * NKH].astype(bf)  # [D, NKH]
        xa, xb = _col_halves(xtk)
        in_maps.append(
            {"xTa": xa, "xTb": xb, "mTa": ma, "mTb": mb, "wvT": wv}
        )
    return in_maps


def _install_ntff_hook_shim():
    """The container's `antenv` stub lacks axon_hooks; register an equivalent
    built on trn_agent_boot's ctypes NTFF driver so trace=True works."""
    import sys
    import types

    if "antenv.axon_hooks" in sys.modules:
        return
    try:
        from trn_agent_boot.trn_boot import _ntff_profile_via_ctypes

        hook = _ntff_profile_via_ctypes("/opt/axon/libaxon_pjrt.so")
    except Exception:
        hook = None
    mod = types.ModuleType("antenv.axon_hooks")
    mod.get_axon_ntff_profile_hook = lambda: hook
    sys.modules["antenv.axon_hooks"] = mod


def kernel(x, Wq, Wk, Wv, _trace=False, _trace_cores=None):
    if _trace:
        _install_ntff_hook_shim()
    in_maps = make_in_maps(x, Wq, Wk, Wv)
    nc = _get_module()
    res = run_bass_kernel_spmd(
        nc,
        in_maps,
        core_ids=list(range(NCORES)),
        trace=_trace,
        trace_cores=_trace_cores,
    )
    out = np.empty((B, N, D), dtype=np.float32)
    for b in range(B):
        # Core rows are [own-half | partner-half]: half-0 cores are already
        # in global query order; half-1 cores need their halves swapped.
        r0 = res.results[2 * b]["out"].astype(np.float32)
        r1 = res.results[2 * b + 1]["out"].astype(np.float32)
        r1 = np.concatenate([r1[NKH:], r1[:NKH]], axis=0)
        osum = r0 + r1
        out[b] = osum[:, :D] / osum[:, D : D + 1]
    if _trace:
        return out, res
    return out


# revision 45
# speedup vs baseline: 1.3071x; 1.1005x over previous
"""Single-head attention (B=4, N=2048, D=1024) on 8 Trainium2 NeuronCores.

Sharding: core c handles batch c//2 and KEY half c%2 (its rows also serve
as its own 1024 queries).

Weight-folding ("M-trick"): scores = q.k^T = x (Wq^T Wk) x^T, so the host
precomputes M = Wq^T Wk once (weights-only preprocessing) and the kernel
computes u = x M for its own queries; the key-side operand of the scores
contraction is then raw x, already resident in SBUF.  This removes the
entire K-projection (65536 PE cycles/core) and the Wk input load.

Pair exchange: instead of AllGather-ing the query shards, the pair runs a
bf16 AllReduce(sum) of u and each core reconstructs the partner's shard
as Z - u_own on the Vector engine.  This keeps the program SPMD-symmetric
while letting every core process its OWN queries first (out rows are
[own | partner]; the host reorders), so the collective hides behind
v-projection + own-half scores/AV.  The exchange is split into two
pipelined AllReduces (e-blocks 0-3 / 4-7); measured: mesh begins ~18us
after launch, each 1MB mesh takes ~21-26us, and the halves pipeline
back-to-back on the CC engine.

Inputs stream as four 1MB pieces (x split by key-half, M split by
e-block-half) + wv, in that trigger order: HWDGE aggregate is only ~190
GB/s and per-queue FIFO drains pieces in trigger order, so u(e0-3, it0)
starts when just (xa, ma) have landed (~14us) instead of after all of
x+M (~24us).  This pulls u-proj AND the collective chain ~10us left.

Precision: projections/AV bf16 (fp32 PSUM).  Scores contraction: ALL 8
e-blocks fp8e4 DoubleRow (a DR matmul costs the same 216ns as bf16 per
512-free tile, so the win is 4 instead of 8 matmuls per tile).  The
AllReduce runs in bf16; the subtraction adds ~0.5% noise only to the
partner's u.  numpy sim of the full chain (bit-exact vs hardware at
NBF=2) predicts rel err 1.81e-2; hardware measures 1.83e-2 vs the 2e-2
gate.  exp in fp32 on the scalar engine; unnormalized softmax; partial
outputs bf16 with the softmax denominator folded into output column
1024.  Host combines the key-halves: out = (oA + oB) / (dA + dB).
"""

from contextlib import ExitStack

import ml_dtypes
import numpy as np

import concourse.bass as bass
import concourse.mybir as mybir
import concourse.tile as tile
from concourse.bass_utils import run_bass_kernel_spmd

B, N, D = 4, 2048, 1024
NCORES = 8
P = 128
NQ = N            # total queries per batch
NKH = N // 2      # keys (and own queries) per core
DC = D // P       # 8 contraction chunks
EC = D // P       # 8 embed blocks
JB = NKH // P     # 8 key blocks
F = 512           # matmul moving free dim (one PSUM bank of fp32)
SCALE = 1.0 / np.sqrt(D)
N_WARM_PRE = 26   # dummy matmuls bridging trigger latency + (xa, ma) load
                  # (~14us) while keeping the HAM warm-up window sustained
ECH = EC // 2     # e-blocks per AllReduce half

BF = mybir.dt.bfloat16
F8 = mybir.dt.float8e4
F32 = mybir.dt.float32

REPLICA_GROUPS = [[0, 1], [2, 3], [4, 5], [6, 7]]


def _attention_kernel(ctx, tc, out, xTa, xTb, mTa, mTb, wvT):
    nc = tc.nc

    consts = ctx.enter_context(tc.tile_pool(name="consts", bufs=1))
    psmain = ctx.enter_context(tc.tile_pool(name="psmain", bufs=2, space="PSUM"))
    psav = ctx.enter_context(tc.tile_pool(name="psav", bufs=5, space="PSUM"))
    pswarm = ctx.enter_context(tc.tile_pool(name="pswarm", bufs=1, space="PSUM"))
    # One staging buffer per output store — no reuse, so the per-store guard
    # memset never carries a WAR wait (DVE memset supports only one).
    outp8 = ctx.enter_context(tc.tile_pool(name="outp8", bufs=1))
    outp4 = ctx.enter_context(tc.tile_pool(name="outp4", bufs=1))
    outp3 = ctx.enter_context(tc.tile_pool(name="outp3", bufs=1))
    outp1 = ctx.enter_context(tc.tile_pool(name="outp1", bufs=1))
    small = ctx.enter_context(tc.tile_pool(name="small", bufs=2))
    dram = ctx.enter_context(tc.tile_pool(name="dram", bufs=1, space="DRAM"))

    # Resident SBUF tensors.  x and M are half-major so each half's DMA
    # lands in one contiguous 8KB-per-partition block (big descriptors):
    # x by KEY half (xTk_sb[:, h, c, k]), M by E-BLOCK half.
    xTk_sb = consts.tile([P, 2, DC, F], BF, tag="xTk")
    m_sb = consts.tile([P, 2, DC, F], BF, tag="m")
    wv_sb = consts.tile([P, DC, D], BF, tag="wv")
    uloc_sb = consts.tile([P, EC, NKH], BF, tag="uloc")  # own u, bf16
    uloc_f8 = consts.tile([P, EC, NKH], F8, tag="ulocf8")
    xk_f8 = consts.tile([P, 2, EC, F], F8, tag="xkf8")   # fp8 x (key side)
    z_sb = consts.tile([P, EC, NKH], BF, tag="z")        # AllReduce result
    upart_f8 = consts.tile([P, EC, NKH], F8, tag="upf8")
    v_sb = consts.tile([P, JB, D], BF, tag="v")          # [p, key-block, e]
    pT_sb = consts.tile([P, JB, NQ], BF, tag="pT")       # [p, key-block, query]
    ones_sb = consts.tile([P, 1], BF, tag="ones")

    def xk_bf(c, j):
        # key-side bf16 x slice [128, 128] for key block j, chunk c
        return xTk_sb[:, j // 4, c, (j % 4) * P : (j % 4 + 1) * P]

    def m_lhsT(c, e):
        return m_sb[:, e // 4, c, (e % 4) * P : (e % 4 + 1) * P]

    # DRAM bounce buffers for the two pairwise bf16 AllReduce halves.
    cc_in = [dram.tile([P, ECH * NKH], BF, name=f"cc{h}_in") for h in range(2)]
    cc_out = [dram.tile([P, ECH * NKH], BF, name=f"cc{h}_out") for h in range(2)]

    nc.vector.memset(ones_sb, 1.0)

    # Input DMAs: four 1MB pieces + wv, drained in trigger order by the
    # per-queue FIFOs.  (xa, ma) unlock u(e0-3, it0); xb unlocks it1; mb
    # unlocks e4-7; wv is only needed by v-projection (~42us).
    in_dmas = []
    dma_xa = nc.sync.dma_start(out=xTk_sb[:, 0, :, :], in_=xTa[:, :])
    dma_ma = nc.sync.dma_start(out=m_sb[:, 0, :, :], in_=mTa[:, :])
    dma_xb = nc.sync.dma_start(out=xTk_sb[:, 1, :, :], in_=xTb[:, :])
    dma_mb = nc.sync.dma_start(out=m_sb[:, 1, :, :], in_=mTb[:, :])
    wv_dma = nc.sync.dma_start(out=wv_sb[:, :, :], in_=wvT[:, :])
    in_dmas += [dma_xa, dma_ma, dma_xb, dma_mb, wv_dma]

    def sp_observe(inst, why):
        n = nc.sync.nop(hint="observe")
        tile.add_dep_helper(n.ins, inst.ins, reason=why)

    # Warm/touch PSUM tile in its own bank: nothing ever reads it, so every
    # write is PE-local and touch matmuls carry exactly one (DMA/DVE) wait.
    warm_src = small.tile([P, 640], BF, tag="warm")
    nc.vector.memset(warm_src, 0.0)
    warm_ps = pswarm.tile([P, F], F32, tag="wps")

    def dummy():
        nc.tensor.matmul(
            warm_ps, lhsT=warm_src[:, 0:P], rhs=warm_src[:, P : P + F],
            start=True, stop=True,
        )

    def touch(t):
        # Trivial matmul whose only purpose is to make the PE observe t's
        # producer (single sync wait), so later real matmuls need none.
        return nc.tensor.matmul(
            warm_ps[0:1, 0:1], lhsT=t[:, 0:1], rhs=t[:, 0:1], start=True, stop=True
        )

    # Solid warm-up block bridging until (xa, ma) land (~14us): HAM
    # un-throttles only after a ~3.4us window of SUSTAINED PE activity.
    for _ in range(N_WARM_PRE):
        dummy()
    touch(xTk_sb[:, 0, 0, :])
    touch(m_sb[:, 0, 0, :])

    # Phase 1: uT[e, it] = sum_c M[c, e-blk]^T x[c, it-tile], piece-gated:
    # e0-3 x it0 first (xa+ma), then it1 (xb), then e4-7 (mb).  AllReduce
    # half h launches as soon as its 4 e-blocks are copied.
    cc_in_dmas = []
    ccs = []

    def launch_cc(h):
        d = nc.gpsimd.dma_start(
            out=cc_in[h][:, :], in_=uloc_sb[:, h * ECH : (h + 1) * ECH, :]
        )
        cc = nc.gpsimd.collective_compute(
            "AllReduce",
            mybir.AluOpType.add,
            replica_groups=REPLICA_GROUPS,
            ins=[cc_in[h][:, :].opt()],
            outs=[cc_out[h][:, :].opt()],
        )
        cc_in_dmas.append(d)
        ccs.append(cc)

    def u_tile(e, it):
        ps = psmain.tile([P, F], F32, tag="ps")
        for c in range(DC):
            nc.tensor.matmul(
                ps,
                lhsT=m_lhsT(c, e),
                rhs=xTk_sb[:, it, c, :],
                start=(c == 0),
                stop=(c == DC - 1),
            )
        nc.vector.tensor_copy(out=uloc_sb[:, e, it * F : (it + 1) * F], in_=ps)

    for e in range(ECH):
        u_tile(e, 0)
    touch(xTk_sb[:, 1, 0, :])
    for e in range(ECH):
        u_tile(e, 1)
    launch_cc(0)
    touch(m_sb[:, 1, 0, :])
    for e in range(ECH, EC):
        u_tile(e, 0)
        u_tile(e, 1)
    launch_cc(1)

    # Phase 2: v[j, e] — lhsT = xTk[d, j-blk], rhs = WvT[d, e-tile].
    # The fp8 casts for the scores contraction (x and own-u pieces) are
    # interleaved between the v copies: a monolithic cast placed before
    # v-proj delays its PSUM-reuse waits (measured 2.2us PE stall), and
    # placed after it would stall tile-0's scores.  Tiny touch copies
    # first so the cast pieces carry no DMA waits.
    dve_scratch = small.tile([P, 2], BF, tag="dvescratch")
    for h in range(2):
        nc.vector.tensor_copy(
            out=dve_scratch[0:1, h : h + 1], in_=xTk_sb[0:1, h, 0, 0:1]
        )
    touch(wv_sb[:, 0, :])
    pieces = [("x", h, e) for h in range(2) for e in range(EC)]
    pieces += [("u", 0, e) for e in range(EC)]
    npc = len(pieces)
    pi = 0
    for j in range(JB):
        for et in range(D // F):
            ps = psmain.tile([P, F], F32, tag="ps")
            for c in range(DC):
                nc.tensor.matmul(
                    ps,
                    lhsT=xk_bf(c, j),
                    rhs=wv_sb[:, c, et * F : (et + 1) * F],
                    start=(c == 0),
                    stop=(c == DC - 1),
                )
            nc.vector.tensor_copy(out=v_sb[:, j, et * F : (et + 1) * F], in_=ps)
            for _ in range((npc - pi) // (16 - (2 * j + et))):
                kind, h, e = pieces[pi]
                pi += 1
                if kind == "x":
                    nc.vector.tensor_copy(
                        out=xk_f8[:, h, e, :], in_=xTk_sb[:, h, e, :]
                    )
                else:
                    nc.vector.tensor_copy(
                        out=uloc_f8[:, e, :], in_=uloc_sb[:, e, :]
                    )

    # Read the AllReduce halves back, STAGGERED (the nop serializes hop 1
    # behind hop 0) so hop 0 gets full read bandwidth.  z_sb is fresh, so
    # each read-back's only dependency is its collective — one wait each.
    rb0 = nc.gpsimd.dma_start(out=z_sb[:, 0:ECH, :], in_=cc_out[0][:, :])
    n_rb = nc.gpsimd.nop(hint="observe")
    tile.add_dep_helper(n_rb.ins, rb0.ins, reason="stagger read-back 1")
    rb1 = nc.gpsimd.dma_start(out=z_sb[:, ECH:EC, :], in_=cc_out[1][:, :])

    # Partner reconstruction on DVE: upart = Z - u_own, cast straight to
    # fp8 (DVE computes in fp32 internally).
    sub0 = nc.vector.tensor_sub(
        upart_f8[:, 0:ECH, :], z_sb[:, 0:ECH, :], uloc_sb[:, 0:ECH, :]
    )
    sub1 = nc.vector.tensor_sub(
        upart_f8[:, ECH:EC, :], z_sb[:, ECH:EC, :], uloc_sb[:, ECH:EC, :]
    )

    # Phase 3: fused scores+AV per 512-query tile.  Tiles 0-1 = OWN queries
    # (u from uloc, no exchange dependency), tiles 2-3 = partner queries.
    # scoresT[j, i] = sum_d' xT[d', j] uT[d', i] — all-fp8 DoubleRow, 4
    # matmuls per (j, tile); p = exp(s*SCALE); then out[i, 0:1024] =
    # pT.T @ v with the denominator folded into column 1024.
    outr = out.rearrange("(gg p) e -> p gg e", p=P)   # [P, 16, D+1]
    STORE_AT = {0: 8, 8: 4, 12: 3, 15: 1}             # start ib -> n blocks
    out_dmas = []
    last_exp = last_mm = last_cp = None

    for t in range(2 * (NKH // F)):
        own = t < NKH // F
        itc = t if own else t - NKH // F
        uf8 = uloc_f8 if own else upart_f8
        if t == NKH // F:
            # Absorb the read-back/subtract waits on the PE so the AV
            # START matmuls never carry a second wait.  Order-pinned after
            # tile 1's last AV matmul: without the pin the scheduler hoists
            # these waits ahead of tile 0/1 and stalls the PE on the
            # exchange (measured 6.6us).
            tch0 = touch(upart_f8[:, 0, 0:1])
            tile.add_dep_helper(
                tch0.ins, last_mm.ins, False, reason="pin after tile1 AV"
            )
            tch1 = touch(upart_f8[:, ECH, 0:1])
            tile.add_dep_helper(
                tch1.ins, tch0.ins, False, reason="pin after first touch"
            )
        for j in range(JB):
            ps = psmain.tile([P, F], F32, tag="ps")
            for i, e in enumerate(range(0, EC, 2)):
                nc.tensor.matmul(
                    ps,
                    lhsT=xk_f8[:, j // 4, e : e + 2, (j % 4) * P : (j % 4 + 1) * P],
                    rhs=uf8[:, e : e + 2, itc * F : (itc + 1) * F],
                    start=(i == 0),
                    stop=(i == EC // 2 - 1),
                    perf_mode=mybir.MatmulPerfMode.DoubleRow,
                )
            last_exp = nc.scalar.activation(
                out=pT_sb[:, j, t * F : (t + 1) * F],
                in_=ps,
                func=mybir.ActivationFunctionType.Exp,
                scale=float(SCALE),
            )
        # AV for this tile's 4 query blocks.  Stores are balanced as
        # [t0+t1: 8 blocks][t2: 4][t3: 3][t3: 1] — gpsimd's SWDGE ring
        # supports only ~8 wait-free DMAs total (4 exchange hops + 4
        # stores), and this split leaves a 262KB final drain.
        for g in range(4):
            ib = 4 * t + g
            if ib in STORE_AT:
                ngrp = STORE_AT[ib]
                pool = {8: outp8, 4: outp4, 3: outp3, 1: outp1}[ngrp]
                o_sb = pool.tile([P, ngrp, D + 1], BF, tag="o")
                o_sb_base = ib
                g2 = nc.vector.memset(o_sb[0:1, 0, 0:1], 0.0)
            gl = ib - o_sb_base
            po0 = psav.tile([P, F], F32, tag="po")
            po1 = psav.tile([P, F], F32, tag="po")
            pd = psav.tile([P, F], F32, tag="po")
            for j in range(JB):
                lhsT = pT_sb[:, j, ib * P : (ib + 1) * P]
                nc.tensor.matmul(
                    po0, lhsT=lhsT, rhs=v_sb[:, j, 0:F],
                    start=(j == 0), stop=(j == JB - 1),
                )
                nc.tensor.matmul(
                    po1, lhsT=lhsT, rhs=v_sb[:, j, F : 2 * F],
                    start=(j == 0), stop=(j == JB - 1),
                )
                last_mm = nc.tensor.matmul(
                    pd[:, 0:1], lhsT=lhsT, rhs=ones_sb,
                    start=(j == 0), stop=(j == JB - 1),
                )
            # Denominator copy first: pd's stop-matmul is the group's last
            # PE tick, so this copy's PE wait covers po0/po1 and the po
            # copies need only their (buffer-reuse) DVE wait.
            dcp = nc.vector.tensor_copy(out=o_sb[:, gl, D : D + 1], in_=pd[:, 0:1])
            tile.add_dep_helper(dcp.ins, g2.ins, False, reason="order after guard")
            if ib == 3:
                # Pin sub0/sub1 into EXACT DVE slots (after tile-g2's last
                # copy, before g3's first): placed earlier by the scheduler
                # their read-back wait head-of-line-blocks every later DVE
                # copy and backpressures the PE via PSUM reuse (measured
                # 14.4us); placed later, tile 2 stalls on the exchange.
                tile.add_dep_helper(dcp.ins, sub0.ins, False, reason="sub0 first")
            if ib == 7:
                tile.add_dep_helper(dcp.ins, sub1.ins, False, reason="sub1 first")
            c0 = nc.vector.tensor_copy(out=o_sb[:, gl, 0:F], in_=po0)
            tile.add_dep_helper(c0.ins, dcp.ins, False, reason="order after dcp")
            last_cp = nc.vector.tensor_copy(out=o_sb[:, gl, F : 2 * F], in_=po1)
            tile.add_dep_helper(last_cp.ins, c0.ins, False, reason="order after c0")
            if ib == 2:
                tile.add_dep_helper(
                    sub0.ins, last_cp.ins, False, reason="sub0 after t0g2"
                )
            if ib == 6:
                tile.add_dep_helper(
                    sub1.ins, last_cp.ins, False, reason="sub1 after t1g2"
                )
            if gl == ngrp - 1:
                # gpsimd SWDGE only: the HWDGE queues already ran the input
                # DMAs, and a reused queue adds a ring-lap wait on top of
                # the store's DVE data dep (one wait per instruction).
                out_dmas.append(
                    nc.gpsimd.dma_start(
                        out=outr[:, o_sb_base : o_sb_base + ngrp, :], in_=o_sb
                    )
                )

    for dmad in in_dmas:
        sp_observe(dmad, "observe input DMA on SP")
    for d in cc_in_dmas:
        sp_observe(d, "observe cc bounce-in DMA on SP")
    for dd in out_dmas:
        sp_observe(dd, "observe output DMA on SP")
    sp_observe(rb0, "observe read-back 0 on SP")
    sp_observe(rb1, "observe read-back 1 on SP")
    sp_observe(last_exp, "observe ACT on SP")
    sp_observe(last_mm, "observe PE on SP")
    sp_observe(last_cp, "observe DVE on SP")


def build_attention_module():
    nc = bass.Bass(trn_type="TRN2", target_bir_lowering=False, debug=False)
    xTa = nc.dram_tensor("xTa", [P, DC * F], BF, kind="ExternalInput").ap()
    xTb = nc.dram_tensor("xTb", [P, DC * F], BF, kind="ExternalInput").ap()
    mTa = nc.dram_tensor("mTa", [P, DC * F], BF, kind="ExternalInput").ap()
    mTb = nc.dram_tensor("mTb", [P, DC * F], BF, kind="ExternalInput").ap()
    wvT = nc.dram_tensor("wvT", [P, DC * D], BF, kind="ExternalInput").ap()
    out = nc.dram_tensor("out", [NQ, D + 1], BF, kind="ExternalOutput").ap()
    with tile.TileContext(nc) as tc:
        with ExitStack() as ctx:
            _attention_kernel(ctx, tc, out, xTa, xTb, mTa, mTb, wvT)
    return nc


_module_cache = None


def _get_module():
    global _module_cache
    if _module_cache is None:
        _module_cache = build_attention_module()
    return _module_cache


def _chunk_blocked(a):
    """[D, n] -> [128, 8*n]: partition p's row = [chunk0 | chunk1 | ...],
    one contiguous DRAM row per partition (big DMA descriptors)."""
    d, n = a.shape
    return np.ascontiguousarray(
        a.reshape(DC, P, n).transpose(1, 0, 2).reshape(P, DC * n)
    )


def _col_halves(a):
    """[D, 1024] -> two [128, 8*512] chunk-blocked column halves."""
    r = a.reshape(DC, P, 2 * F)
    return (
        np.ascontiguousarray(r[:, :, 0:F].transpose(1, 0, 2).reshape(P, DC * F)),
        np.ascontiguousarray(r[:, :, F : 2 * F].transpose(1, 0, 2).reshape(P, DC * F)),
    )


def make_in_maps(x, Wq, Wk, Wv):
    bf = ml_dtypes.bfloat16
    x = np.asarray(x, dtype=np.float32)
    wq = np.asarray(Wq, dtype=np.float32)
    wk = np.asarray(Wk, dtype=np.float32)
    ma, mb = _col_halves((wq.T @ wk).astype(bf))          # contraction-first
    wv = _chunk_blocked(np.asarray(Wv, dtype=np.float32).T.astype(bf))
    in_maps = []
    for core in range(NCORES):
        b, half = divmod(core, 2)
        xtk = x[b].T[:, half * NKH : (half + 1) * NKH].astype(bf)  # [D, NKH]
        xa, xb = _col_halves(xtk)
        in_maps.append(
            {"xTa": xa, "xTb": xb, "mTa": ma, "mTb": mb, "wvT": wv}
        )
    return in_maps


def _install_ntff_hook_shim():
    """The container's `antenv` stub lacks axon_hooks; register an equivalent
    built on trn_agent_boot's ctypes NTFF driver so trace=True works."""
    import sys
    import types

    if "antenv.axon_hooks" in sys.modules:
        return
    try:
        from trn_agent_boot.trn_boot import _ntff_profile_via_ctypes

        hook = _ntff_profile_via_ctypes("/opt/axon/libaxon_pjrt.so")
    except Exception:
        hook = None
    mod = types.ModuleType("antenv.axon_hooks")
    mod.get_axon_ntff_profile_hook = lambda: hook
    sys.modules["antenv.axon_hooks"] = mod


def kernel(x, Wq, Wk, Wv, _trace=False, _trace_cores=None):
    if _trace:
        _install_ntff_hook_shim()
    in_maps = make_in_maps(x, Wq, Wk, Wv)
    nc = _get_module()
    res = run_bass_kernel_spmd(
        nc,
        in_maps,
        core_ids=list(range(NCORES)),
        trace=_trace,
        trace_cores=_trace_cores,
    )
    out = np.empty((B, N, D), dtype=np.float32)
    for b in range(B):
        # Core rows are [own-half | partner-half]: half-0 cores are already
        # in global query order; half-1 cores need their halves swapped.
        r0 = res.results[2 * b]["out"].astype(np.float32)
        r1 = res.results[2 * b + 1]["out"].astype(np.float32)
        r1 = np.concatenate([r1[NKH:], r1[:NKH]], axis=0)
        osum = r0 + r1
        out[b] = osum[:, :D] / osum[:, D : D + 1]
    if _trace:
        return out, res
    return out
